# revision 23
# baseline (speedup 1.0000x reference)
"""Causal multi-head attention block (QKV proj + causal softmax attention +
output proj) for B=4, S=2048, D=1024, H=16 on 8 Trainium2 NeuronCores.

Sharding: core c -> (batch b = c//2, head-group hg = c%2) with 8 heads per
group.  Each core runs an identical Bass/Tile program on its shard; the two
half-head output-projection partials per batch are summed on the host.
"""

import sys

sys.path.insert(0, "/opt/trn_rl_repo")

import numpy as np
import ml_dtypes

B, S, D, H, HD = 4, 2048, 1024, 16, 64
NCORES = 8
HG = 2  # head groups (tensor-parallel shards per batch)
HPC = H // HG  # heads per core = 8
DH = HPC * HD  # head dims per core = 512
SB = 512  # query block
NQB = S // SB  # 4
NST = S // 128  # 16 s-tiles of 128
NKD = D // 128  # 8 contraction tiles over D

BF16 = ml_dtypes.bfloat16

_prog_cache = {}


def _build(has_battn: bool):
    import concourse.bass as bass  # noqa: F401
    import concourse.tile as tile
    from concourse import bacc, mybir
    from contextlib import ExitStack

    dt = mybir.dt
    BF = dt.bfloat16
    F32 = dt.float32
    Exp = mybir.ActivationFunctionType.Exp
    mult = mybir.AluOpType.mult

    nc = bacc.Bacc("TRN2", target_bir_lowering=False, debug=False, num_devices=NCORES)

    xT_d = nc.dram_tensor("xT", [D, S], BF, kind="ExternalInput").ap()
    wqk_d = nc.dram_tensor("wqk", [8, D, 128], BF, kind="ExternalInput").ap()
    wv_d = nc.dram_tensor("wv", [D, DH], BF, kind="ExternalInput").ap()
    wo_d = nc.dram_tensor("wo", [DH, D], BF, kind="ExternalInput").ap()
    mask_d = nc.dram_tensor("mask", [128, 4 * SB], BF, kind="ExternalInput").ap()
    if has_battn:
        bqk_d = nc.dram_tensor("bqk", [128, 8], F32, kind="ExternalInput").ap()
        bv_d = nc.dram_tensor("bv", [128, 4], F32, kind="ExternalInput").ap()
    out_d = nc.dram_tensor("out", [S, D], F32, kind="ExternalOutput").ap()

    with tile.TileContext(nc) as tc, ExitStack() as ctx:
        persist = ctx.enter_context(tc.tile_pool(name="persist", bufs=1))
        ps_sc = ctx.enter_context(tc.tile_pool(name="pssc", bufs=1, space="PSUM"))
        import itertools as _it
        _sc_rr = _it.count()

        def sc_tile(shape, name):
            return ps_sc.tile(shape, F32, tag=f"sc{next(_sc_rr) % 3}", name=name)
        ps_po = ctx.enter_context(tc.tile_pool(name="pspo", bufs=1, space="PSUM"))
        epool = ctx.enter_context(tc.tile_pool(name="ep", bufs=4))
        opool = ctx.enter_context(tc.tile_pool(name="op", bufs=3))
        rpool = ctx.enter_context(tc.tile_pool(name="rp", bufs=2))

        xt = [persist.tile([128, S], BF, tag=f"xt{i}", name=f"xt{i}") for i in range(NKD)]
        wqk = [persist.tile([128, NKD * 128], BF, tag=f"wqk{i}", name=f"wqk{i}") for i in range(8)]
        wv = [persist.tile([128, DH], BF, tag=f"wv{i}", name=f"wv{i}") for i in range(NKD)]
        wo = [persist.tile([128, D], BF, tag=f"wo{i}", name=f"wo{i}") for i in range(DH // 128)]
        qkt = [persist.tile([128, S], BF, tag=f"qkt{m}", name=f"qkt{m}") for m in range(8)]
        v1 = [persist.tile([128, HPC * (HD + 1)], BF, tag=f"v1_{t}", name=f"v1_{t}") for t in range(NST)]
        attn = [persist.tile([128, S], BF, tag=f"attn{t}", name=f"attn{t}") for t in range(4)]
        mask_t = persist.tile([128, 4 * SB], BF, tag="mask", name="mask_t")
        ones_t = persist.tile([1, 64], BF, tag="ones", name="ones_t")
        if has_battn:
            bqk_t = persist.tile([128, 8], F32, tag="bqk", name="bqk_t")
            bv_t = persist.tile([128, 4], F32, tag="bv", name="bv_t")

        # ---- loads (interleaved so first proj matmuls can start early) ----
        for i in range(NKD):
            nc.sync.dma_start(xt[i][:], xT_d[i * 128 : (i + 1) * 128, :])
        for i in range(NKD):
            nc.sync.dma_start(wv[i][:], wv_d[i * 128 : (i + 1) * 128, :])
        for m in (0, 4, 1, 5, 2, 6, 3, 7):
            # wqk chunk m: [1024, 128] -> sbuf [128, 8*128] (k-tiles along free)
            nc.sync.dma_start(
                wqk[m][:].rearrange("p (a c) -> p a c", c=128),
                wqk_d[m].rearrange("(a p) c -> p a c", p=128),
            )
        nc.sync.dma_start(mask_t[:], mask_d[:])
        for i in range(DH // 128):
            nc.sync.dma_start(wo[i][:], wo_d[i * 128 : (i + 1) * 128, :])
        if has_battn:
            nc.sync.dma_start(bqk_t[:], bqk_d[:])
            nc.sync.dma_start(bv_t[:], bv_d[:])
        nc.gpsimd.memset(ones_t[:], 1.0)
        for t in range(NST):
            # ones column per head (col HD of each 65-wide head slot)
            nc.gpsimd.memset(
                v1[t][:].rearrange("p (h c) -> p h c", c=HD + 1)[:, :, HD : HD + 1], 1.0
            )

        # ---- projections -------------------------------------------------
        def v_proj(st):
            ps = sc_tile([128, DH], "psv")
            for k in range(NKD):
                nc.tensor.matmul(
                    ps[:],
                    lhsT=xt[k][:, st * 128 : (st + 1) * 128],
                    rhs=wv[k][:],
                    start=(k == 0),
                    stop=(k == NKD - 1),
                )
            dst = v1[st][:].rearrange("p (h c) -> p h c", c=HD + 1)[:, :, 0:HD]
            nc.scalar.copy(dst, ps[:].rearrange("p (h c) -> p h c", c=HD))

        def qk_proj(m):
            for sb in range(NQB):
                ps = sc_tile([128, SB], "psqk")
                for k in range(NKD):
                    nc.tensor.matmul(
                        ps[:],
                        lhsT=wqk[m][:, k * 128 : (k + 1) * 128],
                        rhs=xt[k][:, sb * SB : (sb + 1) * SB],
                        start=(k == 0),
                        stop=(k == NKD - 1),
                    )
                dst = qkt[m][:, sb * SB : (sb + 1) * SB]
                if has_battn:
                    nc.vector.tensor_scalar_add(dst, ps[:], bqk_t[:, m : m + 1])
                else:
                    nc.scalar.copy(dst, ps[:])

        # ---- attention ---------------------------------------------------
        def attention(p, qb):
            qt, ktile = qkt[p], qkt[4 + p]
            n_kt = 4 * (qb + 1)
            n_g = n_kt // 2
            poA = ps_po.tile([128, SB], F32, tag="poA", name="poA")
            poB = ps_po.tile([128, SB], F32, tag="poB", name="poB")
            for g in range(n_g):
                scA = sc_tile([128, 2 * SB], "scA")
                scB = sc_tile([128, 2 * SB], "scB")
                for j in (0, 1):
                    k = 2 * g + j
                    nc.tensor.matmul(
                        scA[:, j * SB : (j + 1) * SB],
                        lhsT=ktile[0:64, k * 128 : (k + 1) * 128],
                        rhs=qt[0:64, qb * SB : (qb + 1) * SB],
                        start=True, stop=True,
                    )
                    nc.tensor.matmul(
                        scB[:, j * SB : (j + 1) * SB],
                        lhsT=ktile[64:128, k * 128 : (k + 1) * 128],
                        rhs=qt[64:128, qb * SB : (qb + 1) * SB],
                        start=True, stop=True,
                    )
                eA = epool.tile([128, 2 * SB], BF, tag="eA", name="eA")
                eB = epool.tile([128, 2 * SB], BF, tag="eB", name="eB")
                nc.scalar.activation(eA[:], scA[:], Exp, scale=0.125)
                nc.scalar.activation(eB[:], scB[:], Exp, scale=0.125)
                dg = g - (n_g - 2)
                if dg >= 0:
                    msl = mask_t[:, dg * 2 * SB : (dg + 1) * 2 * SB]
                    nc.vector.tensor_tensor(eA[:], eA[:], msl, op=mult)
                    nc.vector.tensor_tensor(eB[:], eB[:], msl, op=mult)
                for j in (0, 1):
                    k = 2 * g + j
                    nc.tensor.matmul(
                        poA[0:65, :],
                        lhsT=v1[k][:, (2 * p) * (HD + 1) : (2 * p) * (HD + 1) + HD + 1],
                        rhs=eA[:, j * SB : (j + 1) * SB],
                        start=(k == 0), stop=(k == n_kt - 1),
                    )
                    nc.tensor.matmul(
                        poB[0:65, :],
                        lhsT=v1[k][:, (2 * p + 1) * (HD + 1) : (2 * p + 1) * (HD + 1) + HD + 1],
                        rhs=eB[:, j * SB : (j + 1) * SB],
                        start=(k == 0), stop=(k == n_kt - 1),
                    )
            for po, off in ((poA, 0), (poB, 64)):
                asl = attn[p][off : off + 64, qb * SB : (qb + 1) * SB]
                nc.vector.tensor_copy(asl, po[0:64, :])
                se = rpool.tile([1, SB], F32, tag="se", name="se")
                nc.vector.tensor_copy(se[:], po[64:65, :])
                r = rpool.tile([1, SB], F32, tag="r", name="r")
                nc.vector.reciprocal_approx_fast(r[:], se[:])
                rb = rpool.tile([1, SB], BF, tag="rb", name="rb")
                nc.vector.tensor_copy(rb[:], r[:])
                bc = sc_tile([128, SB], "bc")
                nc.tensor.matmul(bc[0:64, :], lhsT=ones_t[:], rhs=rb[:], start=True, stop=True)
                nc.vector.tensor_tensor(asl, asl, bc[0:64, :], op=mult)
                if has_battn:
                    nc.scalar.add(asl, asl, bv_t[off : off + 64, p : p + 1])

        def out_proj(st):
            for nb in range(2):
                ps = sc_tile([128, SB], "pso")
                for k in range(4):
                    nc.tensor.matmul(
                        ps[:],
                        lhsT=attn[k][:, st * 128 : (st + 1) * 128],
                        rhs=wo[k][:, nb * SB : (nb + 1) * SB],
                        start=(k == 0),
                        stop=(k == 3),
                    )
                ob = opool.tile([128, SB], F32, tag="ob", name="ob")
                nc.scalar.copy(ob[:], ps[:])
                nc.sync.dma_start(
                    out_d[st * 128 : (st + 1) * 128, nb * SB : (nb + 1) * SB], ob[:]
                )

        for st in range(4):
            v_proj(st)
        for p in range(4):
            qk_proj(p)      # q cols for heads 2p, 2p+1
            qk_proj(4 + p)  # k cols for heads 2p, 2p+1
            attention(p, 0)
        for st in range(4):
            out_proj(st)
        for qb in range(1, NQB):
            for st in range(qb * 4, qb * 4 + 4):
                v_proj(st)
            for p in range(4):
                attention(p, qb)
            for st in range(qb * 4, qb * 4 + 4):
                out_proj(st)

    nc.compile()
    return nc


def _prepare_in_maps(x, W_attn, b_attn, W_out, has_battn):
    q_i = np.arange(128)[:, None]
    q_j = np.arange(SB)[None, :]
    mask = np.zeros((128, 4 * SB), dtype=BF16)
    for pat in range(4):
        mask[:, pat * SB : (pat + 1) * SB] = (pat * 128 + q_i <= q_j).astype(BF16)

    in_maps = []
    for c in range(NCORES):
        b, hg = c // 2, c % 2
        m = {
            "xT": np.ascontiguousarray(x[b].T).astype(BF16),
            "wqk": np.ascontiguousarray(
                np.concatenate(
                    [
                        W_attn[:, hg * DH : (hg + 1) * DH],
                        W_attn[:, D + hg * DH : D + (hg + 1) * DH],
                    ],
                    axis=1,
                )
                .astype(BF16)
                .reshape(D, 8, 128)
                .transpose(1, 0, 2)
            ),
            "wv": W_attn[:, 2 * D + hg * DH : 2 * D + (hg + 1) * DH].astype(BF16),
            "wo": np.ascontiguousarray(W_out[hg * DH : (hg + 1) * DH, :]).astype(BF16),
            "mask": mask,
        }
        if has_battn:
            bq = b_attn[hg * DH : (hg + 1) * DH]
            bk = b_attn[D + hg * DH : D + (hg + 1) * DH]
            bv = b_attn[2 * D + hg * DH : 2 * D + (hg + 1) * DH]
            m["bqk"] = np.ascontiguousarray(
                np.concatenate([bq, bk]).reshape(8, 128).T
            ).astype(np.float32)
            m["bv"] = np.ascontiguousarray(bv.reshape(4, 128).T).astype(np.float32)
        in_maps.append(m)
    return in_maps


def _run(x, W_attn, b_attn, W_out, b_out, trace=False, trace_kwargs=None):
    from concourse.bass_utils import run_bass_kernel_spmd

    x = np.asarray(x, dtype=np.float32)
    W_attn = np.asarray(W_attn, dtype=np.float32)
    b_attn = np.asarray(b_attn, dtype=np.float32)
    W_out = np.asarray(W_out, dtype=np.float32)
    b_out = np.asarray(b_out, dtype=np.float32)

    has_battn = bool(np.any(b_attn != 0.0))
    if has_battn not in _prog_cache:
        _prog_cache[has_battn] = _build(has_battn)
    nc = _prog_cache[has_battn]

    in_maps = _prepare_in_maps(x, W_attn, b_attn, W_out, has_battn)
    res = run_bass_kernel_spmd(
        nc, in_maps, list(range(NCORES)), trace=trace, **(trace_kwargs or {})
    )

    out = np.empty((B, S, D), dtype=np.float32)
    for b in range(B):
        out[b] = res.results[2 * b]["out"] + res.results[2 * b + 1]["out"]
    if np.any(b_out != 0.0):
        out += b_out[None, None, :]
    return out, res


def kernel(x, W_attn, b_attn, W_out, b_out):
    out, _ = _run(x, W_attn, b_attn, W_out, b_out)
    return out


# revision 24
# speedup vs baseline: 1.2363x; 1.2363x over previous
"""Causal multi-head attention block (QKV proj + causal softmax attention +
output proj) for B=4, S=2048, D=1024, H=16 on 8 Trainium2 NeuronCores.

Sharding: core c -> (batch b = c//2, head-group hg = c%2) with 8 heads per
group.  Each core runs an identical Bass/Tile program on its shard; the two
half-head output-projection partials per batch are summed on the host.
"""

import sys

sys.path.insert(0, "/opt/trn_rl_repo")

import numpy as np
import ml_dtypes

B, S, D, H, HD = 4, 2048, 1024, 16, 64
NCORES = 8
HG = 2  # head groups (tensor-parallel shards per batch)
HPC = H // HG  # heads per core = 8
DH = HPC * HD  # head dims per core = 512
SB = 512  # query block
NQB = S // SB  # 4
NST = S // 128  # 16 s-tiles of 128
NKD = D // 128  # 8 contraction tiles over D

BF16 = ml_dtypes.bfloat16

_prog_cache = {}


def _build(has_battn: bool):
    import concourse.bass as bass  # noqa: F401
    import concourse.tile as tile
    from concourse import bacc, mybir
    from contextlib import ExitStack

    dt = mybir.dt
    BF = dt.bfloat16
    F32 = dt.float32
    Exp = mybir.ActivationFunctionType.Exp
    mult = mybir.AluOpType.mult

    nc = bacc.Bacc("TRN2", target_bir_lowering=False, debug=False, num_devices=NCORES)

    xT_d = nc.dram_tensor("xT", [D, S], BF, kind="ExternalInput").ap()
    wqk_d = nc.dram_tensor("wqk", [8, D, 128], BF, kind="ExternalInput").ap()
    wv_d = nc.dram_tensor("wv", [D, DH], BF, kind="ExternalInput").ap()
    wo_d = nc.dram_tensor("wo", [DH, D], BF, kind="ExternalInput").ap()
    mask_d = nc.dram_tensor("mask", [128, 4 * SB], BF, kind="ExternalInput").ap()
    if has_battn:
        bqk_d = nc.dram_tensor("bqk", [128, 8], F32, kind="ExternalInput").ap()
        bv_d = nc.dram_tensor("bv", [128, 4], F32, kind="ExternalInput").ap()
    out_d = nc.dram_tensor("out", [S, D], F32, kind="ExternalOutput").ap()

    with tile.TileContext(nc) as tc, ExitStack() as ctx:
        persist = ctx.enter_context(tc.tile_pool(name="persist", bufs=1))
        ps_mm = ctx.enter_context(tc.tile_pool(name="psmm", bufs=2, space="PSUM"))
        ps_sc = ctx.enter_context(tc.tile_pool(name="pssc", bufs=1, space="PSUM"))
        ps_po = ctx.enter_context(tc.tile_pool(name="pspo", bufs=1, space="PSUM"))
        epool = ctx.enter_context(tc.tile_pool(name="ep", bufs=4))
        opool = ctx.enter_context(tc.tile_pool(name="op", bufs=3))
        rpool = ctx.enter_context(tc.tile_pool(name="rp", bufs=2))

        xt = [persist.tile([128, S], BF, tag=f"xt{i}", name=f"xt{i}") for i in range(NKD)]
        wqk = [persist.tile([128, NKD * 128], BF, tag=f"wqk{i}", name=f"wqk{i}") for i in range(8)]
        wv = [persist.tile([128, DH], BF, tag=f"wv{i}", name=f"wv{i}") for i in range(NKD)]
        wo = [persist.tile([128, D], BF, tag=f"wo{i}", name=f"wo{i}") for i in range(DH // 128)]
        qkt = [persist.tile([128, S], BF, tag=f"qkt{m}", name=f"qkt{m}") for m in range(8)]
        v1 = [persist.tile([128, HPC * (HD + 1)], BF, tag=f"v1_{t}", name=f"v1_{t}") for t in range(NST)]
        attn = [persist.tile([128, S], BF, tag=f"attn{t}", name=f"attn{t}") for t in range(4)]
        mask_t = persist.tile([128, 4 * SB], BF, tag="mask", name="mask_t")
        ones_t = persist.tile([1, 64], BF, tag="ones", name="ones_t")
        if has_battn:
            bqk_t = persist.tile([128, 8], F32, tag="bqk", name="bqk_t")
            bv_t = persist.tile([128, 4], F32, tag="bv", name="bv_t")

        # ---- loads (interleaved so first proj matmuls can start early) ----
        for i in range(NKD):
            nc.sync.dma_start(xt[i][:], xT_d[i * 128 : (i + 1) * 128, :])
        for i in range(NKD):
            nc.sync.dma_start(wv[i][:], wv_d[i * 128 : (i + 1) * 128, :])
        for m in (0, 4, 1, 5, 2, 6, 3, 7):
            # wqk chunk m: [1024, 128] -> sbuf [128, 8*128] (k-tiles along free)
            nc.sync.dma_start(
                wqk[m][:].rearrange("p (a c) -> p a c", c=128),
                wqk_d[m].rearrange("(a p) c -> p a c", p=128),
            )
        nc.sync.dma_start(mask_t[:], mask_d[:])
        for i in range(DH // 128):
            nc.sync.dma_start(wo[i][:], wo_d[i * 128 : (i + 1) * 128, :])
        if has_battn:
            nc.sync.dma_start(bqk_t[:], bqk_d[:])
            nc.sync.dma_start(bv_t[:], bv_d[:])
        nc.gpsimd.memset(ones_t[:], 1.0)
        for t in range(NST):
            # ones column per head (col HD of each 65-wide head slot)
            nc.gpsimd.memset(
                v1[t][:].rearrange("p (h c) -> p h c", c=HD + 1)[:, :, HD : HD + 1], 1.0
            )

        # ---- projections -------------------------------------------------
        def v_proj(st):
            ps = ps_mm.tile([128, DH], F32, tag="mm", name="psv")
            for k in range(NKD):
                nc.tensor.matmul(
                    ps[:],
                    lhsT=xt[k][:, st * 128 : (st + 1) * 128],
                    rhs=wv[k][:],
                    start=(k == 0),
                    stop=(k == NKD - 1),
                )
            dst = v1[st][:].rearrange("p (h c) -> p h c", c=HD + 1)[:, :, 0:HD]
            nc.scalar.copy(dst, ps[:].rearrange("p (h c) -> p h c", c=HD))

        def qk_proj(m):
            for sb in range(NQB):
                ps = ps_mm.tile([128, SB], F32, tag="mm", name="psqk")
                for k in range(NKD):
                    nc.tensor.matmul(
                        ps[:],
                        lhsT=wqk[m][:, k * 128 : (k + 1) * 128],
                        rhs=xt[k][:, sb * SB : (sb + 1) * SB],
                        start=(k == 0),
                        stop=(k == NKD - 1),
                    )
                dst = qkt[m][:, sb * SB : (sb + 1) * SB]
                if has_battn:
                    nc.vector.tensor_scalar_add(dst, ps[:], bqk_t[:, m : m + 1])
                else:
                    nc.scalar.copy(dst, ps[:])

        # ---- attention ---------------------------------------------------
        def attention(p, qb):
            qt, ktile = qkt[p], qkt[4 + p]
            n_kt = 4 * (qb + 1)
            n_g = n_kt // 2
            poA = ps_po.tile([128, SB], F32, tag="poA", name="poA")
            poB = ps_po.tile([128, SB], F32, tag="poB", name="poB")
            for g in range(n_g):
                scA = ps_sc.tile([128, 2 * SB], F32, tag="scA", name="scA")
                scB = ps_sc.tile([128, 2 * SB], F32, tag="scB", name="scB")
                for j in (0, 1):
                    k = 2 * g + j
                    nc.tensor.matmul(
                        scA[:, j * SB : (j + 1) * SB],
                        lhsT=ktile[0:64, k * 128 : (k + 1) * 128],
                        rhs=qt[0:64, qb * SB : (qb + 1) * SB],
                        start=True, stop=True,
                    )
                    nc.tensor.matmul(
                        scB[:, j * SB : (j + 1) * SB],
                        lhsT=ktile[64:128, k * 128 : (k + 1) * 128],
                        rhs=qt[64:128, qb * SB : (qb + 1) * SB],
                        start=True, stop=True,
                    )
                eA = epool.tile([128, 2 * SB], BF, tag="eA", name="eA")
                eB = epool.tile([128, 2 * SB], BF, tag="eB", name="eB")
                nc.scalar.activation(eA[:], scA[:], Exp, scale=0.125)
                nc.scalar.activation(eB[:], scB[:], Exp, scale=0.125)
                dg = g - (n_g - 2)
                if dg >= 0:
                    msl = mask_t[:, dg * 2 * SB : (dg + 1) * 2 * SB]
                    nc.vector.tensor_tensor(eA[:], eA[:], msl, op=mult)
                    nc.vector.tensor_tensor(eB[:], eB[:], msl, op=mult)
                for j in (0, 1):
                    k = 2 * g + j
                    nc.tensor.matmul(
                        poA[0:65, :],
                        lhsT=v1[k][:, (2 * p) * (HD + 1) : (2 * p) * (HD + 1) + HD + 1],
                        rhs=eA[:, j * SB : (j + 1) * SB],
                        start=(k == 0), stop=(k == n_kt - 1),
                    )
                    nc.tensor.matmul(
                        poB[0:65, :],
                        lhsT=v1[k][:, (2 * p + 1) * (HD + 1) : (2 * p + 1) * (HD + 1) + HD + 1],
                        rhs=eB[:, j * SB : (j + 1) * SB],
                        start=(k == 0), stop=(k == n_kt - 1),
                    )
            for po, off in ((poA, 0), (poB, 64)):
                asl = attn[p][off : off + 64, qb * SB : (qb + 1) * SB]
                nc.vector.tensor_copy(asl, po[0:64, :])
                se = rpool.tile([1, SB], F32, tag="se", name="se")
                nc.vector.tensor_copy(se[:], po[64:65, :])
                r = rpool.tile([1, SB], F32, tag="r", name="r")
                nc.vector.reciprocal_approx_fast(r[:], se[:])
                rb = rpool.tile([1, SB], BF, tag="rb", name="rb")
                nc.vector.tensor_copy(rb[:], r[:])
                bc = ps_mm.tile([128, SB], F32, tag="mm", name="bc")
                nc.tensor.matmul(bc[0:64, :], lhsT=ones_t[:], rhs=rb[:], start=True, stop=True)
                nc.vector.tensor_tensor(asl, asl, bc[0:64, :], op=mult)
                if has_battn:
                    nc.scalar.add(asl, asl, bv_t[off : off + 64, p : p + 1])

        def out_proj(st):
            for nb in range(2):
                ps = ps_mm.tile([128, SB], F32, tag="mm", name="pso")
                for k in range(4):
                    nc.tensor.matmul(
                        ps[:],
                        lhsT=attn[k][:, st * 128 : (st + 1) * 128],
                        rhs=wo[k][:, nb * SB : (nb + 1) * SB],
                        start=(k == 0),
                        stop=(k == 3),
                    )
                ob = opool.tile([128, SB], F32, tag="ob", name="ob")
                nc.scalar.copy(ob[:], ps[:])
                nc.sync.dma_start(
                    out_d[st * 128 : (st + 1) * 128, nb * SB : (nb + 1) * SB], ob[:]
                )

        for st in range(4):
            v_proj(st)
        for p in range(4):
            qk_proj(p)      # q cols for heads 2p, 2p+1
            qk_proj(4 + p)  # k cols for heads 2p, 2p+1
            attention(p, 0)
        for st in range(4):
            out_proj(st)
        for qb in range(1, NQB):
            for st in range(qb * 4, qb * 4 + 4):
                v_proj(st)
            for p in range(4):
                attention(p, qb)
            for st in range(qb * 4, qb * 4 + 4):
                out_proj(st)

    nc.compile()
    return nc


def _prepare_in_maps(x, W_attn, b_attn, W_out, has_battn):
    q_i = np.arange(128)[:, None]
    q_j = np.arange(SB)[None, :]
    mask = np.zeros((128, 4 * SB), dtype=BF16)
    for pat in range(4):
        mask[:, pat * SB : (pat + 1) * SB] = (pat * 128 + q_i <= q_j).astype(BF16)

    in_maps = []
    for c in range(NCORES):
        b, hg = c // 2, c % 2
        m = {
            "xT": np.ascontiguousarray(x[b].T).astype(BF16),
            "wqk": np.ascontiguousarray(
                np.concatenate(
                    [
                        W_attn[:, hg * DH : (hg + 1) * DH],
                        W_attn[:, D + hg * DH : D + (hg + 1) * DH],
                    ],
                    axis=1,
                )
                .astype(BF16)
                .reshape(D, 8, 128)
                .transpose(1, 0, 2)
            ),
            "wv": W_attn[:, 2 * D + hg * DH : 2 * D + (hg + 1) * DH].astype(BF16),
            "wo": np.ascontiguousarray(W_out[hg * DH : (hg + 1) * DH, :]).astype(BF16),
            "mask": mask,
        }
        if has_battn:
            bq = b_attn[hg * DH : (hg + 1) * DH]
            bk = b_attn[D + hg * DH : D + (hg + 1) * DH]
            bv = b_attn[2 * D + hg * DH : 2 * D + (hg + 1) * DH]
            m["bqk"] = np.ascontiguousarray(
                np.concatenate([bq, bk]).reshape(8, 128).T
            ).astype(np.float32)
            m["bv"] = np.ascontiguousarray(bv.reshape(4, 128).T).astype(np.float32)
        in_maps.append(m)
    return in_maps


def _run(x, W_attn, b_attn, W_out, b_out, trace=False, trace_kwargs=None):
    from concourse.bass_utils import run_bass_kernel_spmd

    x = np.asarray(x, dtype=np.float32)
    W_attn = np.asarray(W_attn, dtype=np.float32)
    b_attn = np.asarray(b_attn, dtype=np.float32)
    W_out = np.asarray(W_out, dtype=np.float32)
    b_out = np.asarray(b_out, dtype=np.float32)

    has_battn = bool(np.any(b_attn != 0.0))
    if has_battn not in _prog_cache:
        _prog_cache[has_battn] = _build(has_battn)
    nc = _prog_cache[has_battn]

    in_maps = _prepare_in_maps(x, W_attn, b_attn, W_out, has_battn)
    res = run_bass_kernel_spmd(
        nc, in_maps, list(range(NCORES)), trace=trace, **(trace_kwargs or {})
    )

    out = np.empty((B, S, D), dtype=np.float32)
    for b in range(B):
        out[b] = res.results[2 * b]["out"] + res.results[2 * b + 1]["out"]
    if np.any(b_out != 0.0):
        out += b_out[None, None, :]
    return out, res


def kernel(x, W_attn, b_attn, W_out, b_out):
    out, _ = _run(x, W_attn, b_attn, W_out, b_out)
    return out


# revision 26
# speedup vs baseline: 1.2666x; 1.0245x over previous
"""Causal multi-head attention block (QKV proj + causal softmax attention +
output proj) for B=4, S=2048, D=1024, H=16 on 8 Trainium2 NeuronCores.

Sharding: core c -> (batch b = c//2, head-group hg = c%2) with 8 heads per
group.  Each core runs an identical Bass/Tile program on its shard; the two
half-head output-projection partials per batch are summed on the host.
"""

import sys

sys.path.insert(0, "/opt/trn_rl_repo")

import numpy as np
import ml_dtypes

B, S, D, H, HD = 4, 2048, 1024, 16, 64
NCORES = 8
HG = 2  # head groups (tensor-parallel shards per batch)
HPC = H // HG  # heads per core = 8
DH = HPC * HD  # head dims per core = 512
SB = 512  # query block
NQB = S // SB  # 4
NST = S // 128  # 16 s-tiles of 128
NKD = D // 128  # 8 contraction tiles over D

BF16 = ml_dtypes.bfloat16

_prog_cache = {}


def _build(has_battn: bool):
    import concourse.bass as bass  # noqa: F401
    import concourse.tile as tile
    from concourse import bacc, mybir
    from contextlib import ExitStack

    dt = mybir.dt
    BF = dt.bfloat16
    F32 = dt.float32
    Exp = mybir.ActivationFunctionType.Exp
    mult = mybir.AluOpType.mult

    nc = bacc.Bacc("TRN2", target_bir_lowering=False, debug=False, num_devices=NCORES)

    xT_d = nc.dram_tensor("xT", [D, S], BF, kind="ExternalInput").ap()
    wqk_d = nc.dram_tensor("wqk", [8, D, 128], BF, kind="ExternalInput").ap()
    wv_d = nc.dram_tensor("wv", [D, DH], BF, kind="ExternalInput").ap()
    wo_d = nc.dram_tensor("wo", [DH, D], BF, kind="ExternalInput").ap()
    mask_d = nc.dram_tensor("mask", [128, 4 * SB], BF, kind="ExternalInput").ap()
    if has_battn:
        bqk_d = nc.dram_tensor("bqk", [128, 8], F32, kind="ExternalInput").ap()
        bv_d = nc.dram_tensor("bv", [128, 4], F32, kind="ExternalInput").ap()
    out_d = nc.dram_tensor("out", [S, D], F32, kind="ExternalOutput").ap()

    with tile.TileContext(nc) as tc, ExitStack() as ctx:
        persist = ctx.enter_context(tc.tile_pool(name="persist", bufs=1))
        ps_mm = ctx.enter_context(tc.tile_pool(name="psmm", bufs=2, space="PSUM"))
        ps_sc = ctx.enter_context(tc.tile_pool(name="pssc", bufs=1, space="PSUM"))
        ps_po = ctx.enter_context(tc.tile_pool(name="pspo", bufs=1, space="PSUM"))
        epool = ctx.enter_context(tc.tile_pool(name="ep", bufs=4))
        opool = ctx.enter_context(tc.tile_pool(name="op", bufs=3))
        rpool = ctx.enter_context(tc.tile_pool(name="rp", bufs=2))

        xt = [persist.tile([128, S], BF, tag=f"xt{i}", name=f"xt{i}") for i in range(NKD)]
        wqk = [persist.tile([128, NKD * 128], BF, tag=f"wqk{i}", name=f"wqk{i}") for i in range(8)]
        wv = [persist.tile([128, DH], BF, tag=f"wv{i}", name=f"wv{i}") for i in range(NKD)]
        wo = [persist.tile([128, D], BF, tag=f"wo{i}", name=f"wo{i}") for i in range(DH // 128)]
        qkt = [persist.tile([128, S], BF, tag=f"qkt{m}", name=f"qkt{m}") for m in range(8)]
        v1 = [persist.tile([128, HPC * (HD + 1)], BF, tag=f"v1_{t}", name=f"v1_{t}") for t in range(NST)]
        attn = [persist.tile([128, S], BF, tag=f"attn{t}", name=f"attn{t}") for t in range(4)]
        mask_t = persist.tile([128, 4 * SB], BF, tag="mask", name="mask_t")
        ones_t = persist.tile([1, 64], BF, tag="ones", name="ones_t")
        if has_battn:
            bqk_t = persist.tile([128, 8], F32, tag="bqk", name="bqk_t")
            bv_t = persist.tile([128, 4], F32, tag="bv", name="bv_t")

        # ---- loads (interleaved so first proj matmuls can start early) ----
        for i in range(NKD):
            nc.sync.dma_start(xt[i][:], xT_d[i * 128 : (i + 1) * 128, :])
        for i in range(NKD):
            nc.sync.dma_start(wv[i][:], wv_d[i * 128 : (i + 1) * 128, :])
        for m in (0, 4, 1, 5, 2, 6, 3, 7):
            # wqk chunk m: [1024, 128] -> sbuf [128, 8*128] (k-tiles along free)
            nc.sync.dma_start(
                wqk[m][:].rearrange("p (a c) -> p a c", c=128),
                wqk_d[m].rearrange("(a p) c -> p a c", p=128),
            )
        nc.sync.dma_start(mask_t[:], mask_d[:])
        for i in range(DH // 128):
            nc.sync.dma_start(wo[i][:], wo_d[i * 128 : (i + 1) * 128, :])
        if has_battn:
            nc.sync.dma_start(bqk_t[:], bqk_d[:])
            nc.sync.dma_start(bv_t[:], bv_d[:])
        nc.gpsimd.memset(ones_t[:], 1.0)
        for t in range(NST):
            # ones column per head (col HD of each 65-wide head slot)
            nc.gpsimd.memset(
                v1[t][:].rearrange("p (h c) -> p h c", c=HD + 1)[:, :, HD : HD + 1], 1.0
            )

        # ---- projections -------------------------------------------------
        def v_proj(st):
            ps = ps_mm.tile([128, DH], F32, tag="mm", name="psv")
            for k in range(NKD):
                nc.tensor.matmul(
                    ps[:],
                    lhsT=xt[k][:, st * 128 : (st + 1) * 128],
                    rhs=wv[k][:],
                    start=(k == 0),
                    stop=(k == NKD - 1),
                )
            dst = v1[st][:].rearrange("p (h c) -> p h c", c=HD + 1)[:, :, 0:HD]
            nc.scalar.copy(dst, ps[:].rearrange("p (h c) -> p h c", c=HD))

        def qk_proj(m):
            for sb in range(NQB):
                ps = ps_mm.tile([128, SB], F32, tag="mm", name="psqk")
                for k in range(NKD):
                    nc.tensor.matmul(
                        ps[:],
                        lhsT=wqk[m][:, k * 128 : (k + 1) * 128],
                        rhs=xt[k][:, sb * SB : (sb + 1) * SB],
                        start=(k == 0),
                        stop=(k == NKD - 1),
                    )
                dst = qkt[m][:, sb * SB : (sb + 1) * SB]
                if has_battn:
                    nc.vector.tensor_scalar_add(dst, ps[:], bqk_t[:, m : m + 1])
                else:
                    nc.scalar.copy(dst, ps[:])

        # ---- attention ---------------------------------------------------
        def attention(p, qb):
            qt, ktile = qkt[p], qkt[4 + p]
            n_kt = 4 * (qb + 1)
            n_g = n_kt // 2
            poA = ps_po.tile([128, SB], F32, tag="poA", name="poA")
            poB = ps_po.tile([128, SB], F32, tag="poB", name="poB")
            def qk_head(off, g, tag):
                sc = ps_sc.tile([128, 2 * SB], F32, tag=tag, name="sc" + tag)
                for j in (0, 1):
                    k = 2 * g + j
                    nc.tensor.matmul(
                        sc[:, j * SB : (j + 1) * SB],
                        lhsT=ktile[off : off + 64, k * 128 : (k + 1) * 128],
                        rhs=qt[off : off + 64, qb * SB : (qb + 1) * SB],
                        start=True, stop=True,
                    )
                return sc

            def tail_head(off, g, sc, po, vslot):
                e = epool.tile([128, 2 * SB], BF, tag="eA" if off == 0 else "eB",
                               name="e")
                nc.scalar.activation(e[:], sc[:], Exp, scale=0.125)
                nxt = None
                if g + 1 < n_g:
                    nxt = qk_head(off, g + 1, "scA" if off == 0 else "scB")
                dg = g - (n_g - 2)
                if dg >= 0:
                    msl = mask_t[:, dg * 2 * SB : (dg + 1) * 2 * SB]
                    nc.vector.tensor_tensor(e[:], e[:], msl, op=mult)
                for j in (0, 1):
                    k = 2 * g + j
                    nc.tensor.matmul(
                        po[0:65, :],
                        lhsT=v1[k][:, vslot : vslot + HD + 1],
                        rhs=e[:, j * SB : (j + 1) * SB],
                        start=(k == 0), stop=(k == n_kt - 1),
                    )
                return nxt

            vA = (2 * p) * (HD + 1)
            vB = (2 * p + 1) * (HD + 1)
            scA = qk_head(0, 0, "scA")
            scB = qk_head(64, 0, "scB")
            for g in range(n_g):
                scA = tail_head(0, g, scA, poA, vA)
                scB = tail_head(64, g, scB, poB, vB)
            for po, off in ((poA, 0), (poB, 64)):
                asl = attn[p][off : off + 64, qb * SB : (qb + 1) * SB]
                nc.vector.tensor_copy(asl, po[0:64, :])
                se = rpool.tile([1, SB], F32, tag="se", name="se")
                nc.vector.tensor_copy(se[:], po[64:65, :])
                r = rpool.tile([1, SB], F32, tag="r", name="r")
                nc.vector.reciprocal_approx_fast(r[:], se[:])
                rb = rpool.tile([1, SB], BF, tag="rb", name="rb")
                nc.vector.tensor_copy(rb[:], r[:])
                bc = ps_mm.tile([128, SB], F32, tag="mm", name="bc")
                nc.tensor.matmul(bc[0:64, :], lhsT=ones_t[:], rhs=rb[:], start=True, stop=True)
                nc.vector.tensor_tensor(asl, asl, bc[0:64, :], op=mult)
                if has_battn:
                    nc.scalar.add(asl, asl, bv_t[off : off + 64, p : p + 1])

        def out_proj(st):
            for nb in range(2):
                ps = ps_mm.tile([128, SB], F32, tag="mm", name="pso")
                for k in range(4):
                    nc.tensor.matmul(
                        ps[:],
                        lhsT=attn[k][:, st * 128 : (st + 1) * 128],
                        rhs=wo[k][:, nb * SB : (nb + 1) * SB],
                        start=(k == 0),
                        stop=(k == 3),
                    )
                ob = opool.tile([128, SB], F32, tag="ob", name="ob")
                nc.scalar.copy(ob[:], ps[:])
                nc.sync.dma_start(
                    out_d[st * 128 : (st + 1) * 128, nb * SB : (nb + 1) * SB], ob[:]
                )

        for st in range(4):
            v_proj(st)
        for p in range(4):
            qk_proj(p)      # q cols for heads 2p, 2p+1
            qk_proj(4 + p)  # k cols for heads 2p, 2p+1
            attention(p, 0)
        for st in range(4):
            out_proj(st)
        for qb in range(1, NQB):
            for st in range(qb * 4, qb * 4 + 4):
                v_proj(st)
            for p in range(4):
                attention(p, qb)
            for st in range(qb * 4, qb * 4 + 4):
                out_proj(st)

    nc.compile()
    return nc


def _prepare_in_maps(x, W_attn, b_attn, W_out, has_battn):
    q_i = np.arange(128)[:, None]
    q_j = np.arange(SB)[None, :]
    mask = np.zeros((128, 4 * SB), dtype=BF16)
    for pat in range(4):
        mask[:, pat * SB : (pat + 1) * SB] = (pat * 128 + q_i <= q_j).astype(BF16)

    in_maps = []
    for c in range(NCORES):
        b, hg = c // 2, c % 2
        m = {
            "xT": np.ascontiguousarray(x[b].T).astype(BF16),
            "wqk": np.ascontiguousarray(
                np.concatenate(
                    [
                        W_attn[:, hg * DH : (hg + 1) * DH],
                        W_attn[:, D + hg * DH : D + (hg + 1) * DH],
                    ],
                    axis=1,
                )
                .astype(BF16)
                .reshape(D, 8, 128)
                .transpose(1, 0, 2)
            ),
            "wv": W_attn[:, 2 * D + hg * DH : 2 * D + (hg + 1) * DH].astype(BF16),
            "wo": np.ascontiguousarray(W_out[hg * DH : (hg + 1) * DH, :]).astype(BF16),
            "mask": mask,
        }
        if has_battn:
            bq = b_attn[hg * DH : (hg + 1) * DH]
            bk = b_attn[D + hg * DH : D + (hg + 1) * DH]
            bv = b_attn[2 * D + hg * DH : 2 * D + (hg + 1) * DH]
            m["bqk"] = np.ascontiguousarray(
                np.concatenate([bq, bk]).reshape(8, 128).T
            ).astype(np.float32)
            m["bv"] = np.ascontiguousarray(bv.reshape(4, 128).T).astype(np.float32)
        in_maps.append(m)
    return in_maps


def _run(x, W_attn, b_attn, W_out, b_out, trace=False, trace_kwargs=None):
    from concourse.bass_utils import run_bass_kernel_spmd

    x = np.asarray(x, dtype=np.float32)
    W_attn = np.asarray(W_attn, dtype=np.float32)
    b_attn = np.asarray(b_attn, dtype=np.float32)
    W_out = np.asarray(W_out, dtype=np.float32)
    b_out = np.asarray(b_out, dtype=np.float32)

    has_battn = bool(np.any(b_attn != 0.0))
    if has_battn not in _prog_cache:
        _prog_cache[has_battn] = _build(has_battn)
    nc = _prog_cache[has_battn]

    in_maps = _prepare_in_maps(x, W_attn, b_attn, W_out, has_battn)
    res = run_bass_kernel_spmd(
        nc, in_maps, list(range(NCORES)), trace=trace, **(trace_kwargs or {})
    )

    out = np.empty((B, S, D), dtype=np.float32)
    for b in range(B):
        out[b] = res.results[2 * b]["out"] + res.results[2 * b + 1]["out"]
    if np.any(b_out != 0.0):
        out += b_out[None, None, :]
    return out, res


def kernel(x, W_attn, b_attn, W_out, b_out):
    out, _ = _run(x, W_attn, b_attn, W_out, b_out)
    return out
